# revision 21
# baseline (speedup 1.0000x reference)
# MoE (top-2 of 8 experts) Trainium2 kernel.
#
# Strategy — expert-parallel with pairwise H-split (a refinement of the
# "shard experts across devices, all-to-all dispatch" hint):
#   - Gate (softmax + top-2 + renormalize) computed on host in f32 — it is
#     0.006% of the FLOPs and produces the data-dependent routing needed to
#     shard the tokens.
#   - Experts are sorted by routed-token count: the 4 heaviest and the 4
#     lightest are paired up. Each of the 4 pairs maps onto 2 cores: both
#     cores process BOTH experts' full token batches, but each core computes
#     only half of the hidden dimension H. The host sums the two half-H
#     partials. This caps per-core work at (max_heavy + max_light)/2 token
#     FFNs instead of max_all, which is much closer to the perfect 2048.
#   - Device per token batch: dense FFN  relu(x @ w1h + b1h) @ w2h on the
#     TensorEngine (bf16 inputs, fp32 PSUM accumulation); b2 and the gate
#     combine weights are applied on the host during unshard.
#
# Device kernel layout (all "transposed": tokens on the matmul free dim):
#   phase 1:  hT[mh]  = relu( w1[kc,:,mh*128:..].T @ xT[kc]  summed over kc + b1 )
#   phase 2:  outT[mc] =       w2[kh,:,mc*128:..].T @ hT[kh]  summed over kh
# Both experts' half-H weights stay resident in SBUF (bf16, 16 MB); token
# chunks of <=512 stream through.

import os
import sys
import types

import numpy as np
import ml_dtypes

P = 128
C = 1024
H = 4096
H2 = H // 2
E = 8
N_CORES = 8
KC = C // P     # 8
KH2 = H2 // P   # 16
BF16 = ml_dtypes.bfloat16

TRACE = bool(int(os.environ.get("KERNEL_TRACE", "0")))
LAST_EXEC_NS = None
LAST_RESULTS = None


def _ensure_axon_hooks_shim():
    """bass_utils imports antenv.axon_hooks when tracing is requested; this
    image's antenv lacks that module. Provide it, backed by the axon PJRT .so
    profiling C ABI when available."""
    try:
        import antenv.axon_hooks  # noqa: F401
        return
    except ImportError:
        pass
    mod = types.ModuleType("antenv.axon_hooks")
    mod._hook = None

    def set_axon_ntff_profile_hook(h):
        mod._hook = h

    def get_axon_ntff_profile_hook():
        return mod._hook

    mod.set_axon_ntff_profile_hook = set_axon_ntff_profile_hook
    mod.get_axon_ntff_profile_hook = get_axon_ntff_profile_hook
    try:
        import antenv
        sys.modules["antenv.axon_hooks"] = mod
        antenv.axon_hooks = mod
    except ImportError:
        antenv = types.ModuleType("antenv")
        antenv.axon_hooks = mod
        sys.modules["antenv"] = antenv
        sys.modules["antenv.axon_hooks"] = mod
    try:
        from trn_agent_boot.trn_boot import _ntff_profile_via_ctypes
        h = _ntff_profile_via_ctypes("/opt/axon/libaxon_pjrt.so")
        if h is not None:
            mod._hook = h
    except Exception:
        pass


_COMPILED = {}


def _equal_chunks(cap):
    n = -(-cap // 512)
    q, r = divmod(cap // P, n)
    return [(q + 1) * P] * r + [q * P] * (n - r)


def _first_chunks(cap):
    """Chunk plan for the first-processed segment: a 384-token lead chunk —
    wide enough that phase-1 weight consumption (1.24us/group) stays behind
    the DMA wire ramp, narrow enough to start matmuls after ~1.25 MB."""
    if cap <= 768:
        return _equal_chunks(cap)
    return [384] + _equal_chunks(cap - 384)


# column-block widths for the first-processed segment's w1: a small lead
# block so phase-1 group 0 starts early, then sizes that track the ramp
W1_PLAN_LEAD = [256, 384, 512, 896]
W1_PLAN_STEADY = [512, 512, 512, 512]


def _block_map(plan):
    """mh-group -> (block index, column-within-block) for a block-width plan."""
    out = []
    for b, bw in enumerate(plan):
        for col in range(bw // P):
            out.append((b, col))
    return out


def _build(cap_a, cap_b):
    import concourse.mybir as mybir
    import concourse.tile as tile
    from concourse import bacc

    f32 = mybir.dt.float32
    bf16 = mybir.dt.bfloat16

    nc = bacc.Bacc("TRN2", target_bir_lowering=False, debug=False,
                   num_devices=N_CORES)

    caps = {"a": cap_a, "b": cap_b}
    x_d, w1_d, w2_d, out_d = {}, {}, {}, {}
    for s in ("a", "b"):
        x_d[s] = nc.dram_tensor(f"xt{s}", [C, caps[s]], bf16,
                                kind="ExternalInput")
        w1_d[s] = nc.dram_tensor(f"w1{s}", [C, H2], bf16,
                                 kind="ExternalInput")
        w2_d[s] = nc.dram_tensor(f"w2{s}", [H2, C], bf16,
                                 kind="ExternalInput")
        out_d[s] = nc.dram_tensor(f"out{s}", [C, caps[s]], f32,
                                  kind="ExternalOutput")
    b1_d = nc.dram_tensor("b1r", [P, 2 * KH2], f32, kind="ExternalInput")

    # partition-major views: [p, kc/kh/mc, free] so one DMA covers all
    # 128-row tiles of a tensor (each dma_start trigger costs ~600ns on the
    # Sync sequencer — merged transfers keep the trigger count tiny)
    x_t = {s: x_d[s].ap().rearrange("(kc p) n -> p kc n", p=P)
           for s in ("a", "b")}
    w1_t = {s: w1_d[s].ap().rearrange("(kc p) h -> p kc h", p=P)
            for s in ("a", "b")}
    w2_t = {s: w2_d[s].ap().rearrange("(kh p) c -> p kh c", p=P)
            for s in ("a", "b")}
    out_t = {s: out_d[s].ap().rearrange("(mc p) n -> p mc n", p=P)
             for s in ("a", "b")}

    relu = mybir.ActivationFunctionType.Relu

    # segment "a" (heavy experts) runs first with the narrow lead chunk and
    # lead w1 block — minimal DMA before the first matmul. Putting the lead
    # chunk in the segment whose capacity is NOT a multiple of 512 keeps the
    # total chunk count (and so matmul instruction count) at its minimum.
    SEG_ORDER = ("a", "b")
    chunks = {"a": _first_chunks(cap_a), "b": _equal_chunks(cap_b)}
    w1_plan = {"a": W1_PLAN_LEAD, "b": W1_PLAN_STEADY}
    w1_map = {s: _block_map(w1_plan[s]) for s in ("a", "b")}

    with tile.TileContext(nc) as tc:
        with (
            tc.tile_pool(name="wres", bufs=1) as wpool,
            tc.tile_pool(name="bias", bufs=1) as bpool,
            tc.tile_pool(name="xin", bufs=2) as xpool,
            tc.tile_pool(name="hmid", bufs=1) as hpool,
            tc.tile_pool(name="oout", bufs=1) as opool,
            tc.tile_pool(name="ps1", bufs=4, space="PSUM") as ps1pool,
            tc.tile_pool(name="ps2", bufs=4, space="PSUM") as ps2pool,
        ):
            # bias via GpSimd SWDGE: its trigger runs in parallel with the
            # Sync-side loads; the phase-1 relu (which drains PSUM slots)
            # needs b1 early
            b1_sb = bpool.tile([P, 2 * KH2], f32, tag="b1")
            nc.gpsimd.dma_start(b1_sb[:], b1_d.ap())

            # first token chunk (first segment, 384 tokens): critical path
            W0 = chunks[SEG_ORDER[0]][0]
            x_first = xpool.tile([P, KC * W0], bf16, tag="x")
            nc.sync.dma_start(
                x_first[:].rearrange("p (kc w) -> p kc w", kc=KC),
                x_t[SEG_ORDER[0]][:, :, 0:W0])

            # weights in consumption order: w1b, w2b, w1a, w2a
            w1_sb, w2_sb = {}, {}
            for s in SEG_ORDER:
                w1_sb[s] = []
                hoff = 0
                for blk, bw in enumerate(w1_plan[s]):
                    t = wpool.tile([P, KC * bw], bf16, tag=f"w1{s}_{blk}")
                    nc.sync.dma_start(
                        t[:].rearrange("p (kc w) -> p kc w", kc=KC),
                        w1_t[s][:, :, hoff:hoff + bw])
                    w1_sb[s].append((t, bw // P))
                    hoff += bw
                w2_sb[s] = []
                for mc in range(KC):
                    t = wpool.tile([P, KH2 * P], bf16, tag=f"w2{s}_{mc}")
                    nc.sync.dma_start(
                        t[:].rearrange("p (kh w) -> p kh w", kh=KH2),
                        w2_t[s][:, :, mc * P:(mc + 1) * P])
                    w2_sb[s].append(t)

            b1_seg_off = {"a": 0, "b": KH2}
            for si, s in enumerate(SEG_ORDER):
                b1_off = b1_seg_off[s]
                off = 0
                seg_chunks = chunks[s]
                for ci, W in enumerate(seg_chunks):
                    if si == 0 and ci == 0:
                        x_sb = x_first
                    else:
                        x_sb = xpool.tile([P, KC * W], bf16, tag="x")
                        nc.gpsimd.dma_start(
                            x_sb[:].rearrange("p (kc w) -> p kc w", kc=KC),
                            x_t[s][:, :, off:off + W])

                    h_sb = []
                    for mh in range(KH2):
                        blk, col = w1_map[s][mh]
                        w1t, gpb = w1_sb[s][blk]
                        ps = ps1pool.tile([P, W], f32, tag="ps1")
                        for kc in range(KC):
                            nc.tensor.matmul(
                                ps[:],
                                w1t[:, (kc * gpb + col) * P:
                                    (kc * gpb + col) * P + P],
                                x_sb[:, kc * W:(kc + 1) * W],
                                start=(kc == 0),
                                stop=(kc == KC - 1),
                            )
                        ht = hpool.tile([P, W], bf16, tag=f"h_{mh}")
                        nc.scalar.activation(
                            ht[:], ps[:], relu,
                            bias=b1_sb[:, b1_off + mh:b1_off + mh + 1],
                            scale=1.0)
                        h_sb.append(ht)

                    # last chunk overall: two half-tiles so the first half's
                    # store drains while the last PSUM groups finish
                    last = (si == 1 and ci == len(seg_chunks) - 1)
                    n_osplit = 2 if last else 1
                    mc_per = KC // n_osplit
                    for sp in range(n_osplit):
                        o_sb = opool.tile([P, mc_per * W], f32, tag=f"o_{sp}")
                        for mci in range(mc_per):
                            mc = sp * mc_per + mci
                            ps = ps2pool.tile([P, W], f32, tag="ps2")
                            for kh in range(KH2):
                                nc.tensor.matmul(
                                    ps[:],
                                    w2_sb[s][mc][:, kh * P:(kh + 1) * P],
                                    h_sb[kh][:],
                                    start=(kh == 0),
                                    stop=(kh == KH2 - 1),
                                )
                            nc.vector.tensor_copy(
                                o_sb[:, mci * W:(mci + 1) * W], ps[:])
                        nc.sync.dma_start(
                            out_t[s][:, sp * mc_per:(sp + 1) * mc_per,
                                     off:off + W],
                            o_sb[:].rearrange("p (mc w) -> p mc w", mc=mc_per))
                    off += W

    nc.compile()
    return nc


def _get_compiled(cap_a, cap_b):
    key = (cap_a, cap_b)
    if key not in _COMPILED:
        _COMPILED[key] = _build(cap_a, cap_b)
    return _COMPILED[key]


def kernel(x, gate_w, w1, b1, w2, b2):
    global LAST_EXEC_NS, LAST_RESULTS
    _ensure_axon_hooks_shim()
    from concourse import bass_utils

    B, T, _ = x.shape
    N = B * T
    xf = np.ascontiguousarray(x.reshape(N, C)).astype(np.float32, copy=False)

    # --- gate on host (f32, matches reference numerics) ---
    logits = xf @ np.ascontiguousarray(gate_w.astype(np.float32)).T
    m = logits.max(axis=1, keepdims=True)
    ew = np.exp(logits - m)
    sw = ew / ew.sum(axis=1, keepdims=True)        # [N, E] f32 softmax
    ar = np.arange(N)
    i0 = sw.argmax(axis=1)
    w0 = sw[ar, i0]
    swm = sw.copy()
    swm[ar, i0] = -1.0
    i1 = swm.argmax(axis=1)
    w1g = sw[ar, i1]
    tot = w0 + w1g
    cw0 = (w0 / tot).astype(np.float32)
    cw1 = (w1g / tot).astype(np.float32)

    # --- dispatch: token lists per expert ---
    idx_list, cw_list = [], []
    for e in range(E):
        s0 = i0 == e
        s1 = i1 == e
        idx_list.append(np.concatenate([ar[s0], ar[s1]]))
        cw_list.append(np.concatenate([cw0[s0], cw1[s1]]).astype(np.float32))
    counts = np.array([len(ix) for ix in idx_list])

    # pair heavy experts with light ones; each pair -> 2 cores (H halves)
    order = np.argsort(-counts, kind="stable")
    big4, small4 = order[:4], order[4:]
    cap_a = max(((counts[big4].max() + P - 1) // P) * P, P)
    cap_b = max(((counts[small4].max() + P - 1) // P) * P, P)

    nc = _get_compiled(int(cap_a), int(cap_b))

    # --- per-core inputs ---
    w1b16 = w1.astype(BF16)                                      # [E, C, H]
    w2b16 = w2.astype(BF16)                                      # [E, H, C]
    b1f = b1.astype(np.float32)

    def xt_for(e, cap):
        xt = np.zeros((C, cap), dtype=BF16)
        xt[:, :counts[e]] = np.ascontiguousarray(xf[idx_list[e]].T)
        return xt

    xta = {int(e): xt_for(int(e), int(cap_a)) for e in big4}
    xtb = {int(e): xt_for(int(e), int(cap_b)) for e in small4}

    in_maps = []
    for core in range(N_CORES):
        i, h = divmod(core, 2)
        ea, eb = int(big4[i]), int(small4[i])
        hs = slice(h * H2, (h + 1) * H2)
        b1r = np.concatenate([
            b1f[ea, hs].reshape(KH2, P).T,
            b1f[eb, hs].reshape(KH2, P).T,
        ], axis=1)
        in_maps.append({
            "xta": xta[ea],
            "xtb": xtb[eb],
            "w1a": np.ascontiguousarray(w1b16[ea][:, hs]),
            "w1b": np.ascontiguousarray(w1b16[eb][:, hs]),
            "w2a": np.ascontiguousarray(w2b16[ea][hs, :]),
            "w2b": np.ascontiguousarray(w2b16[eb][hs, :]),
            "b1r": np.ascontiguousarray(b1r),
        })

    try:
        res = bass_utils.run_bass_kernel_spmd(
            nc, in_maps, core_ids=list(range(N_CORES)), trace=TRACE)
    except Exception:
        if not TRACE:
            raise
        # profiling plumbing can fail in restricted environments — the
        # numerical result must not depend on it
        res = bass_utils.run_bass_kernel_spmd(
            nc, in_maps, core_ids=list(range(N_CORES)), trace=False)
    LAST_RESULTS = res
    LAST_EXEC_NS = res.exec_time_ns

    # --- combine (host unshard): sum H-halves, add b2, apply gate weights ---
    out = np.zeros((N, C), dtype=np.float32)
    b2f = b2.astype(np.float32)
    for i in range(4):
        for seg, e_arr in (("a", big4), ("b", small4)):
            e = int(e_arr[i])
            n_e = int(counts[e])
            y = (res.results[2 * i][f"out{seg}"][:, :n_e].T +
                 res.results[2 * i + 1][f"out{seg}"][:, :n_e].T)
            y += b2f[e][None, :]
            out[idx_list[e]] += cw_list[e][:, None] * y
    return out.reshape(B, T, C).astype(x.dtype, copy=False)
